# revision 40
# baseline (speedup 1.0000x reference)
"""Multi-head self-attention (B=2, S=2048, D=1024, H=16, causal+padding mask)
on 8 Trainium2 NeuronCores via Bass/Tile, SPMD.

Sharding: head-split tensor parallelism. Core c -> batch b = c//4, head group
g = c%4 (heads 4g..4g+3, model dims 256g..256g+255). Each core projects only
its 256-dim Q/K/V slice over the full 2048-token batch, runs blocked-causal
attention for its 4 heads, and emits a row-parallel partial output
Y_g^T = Wo[:, g-dims]^T A_g^T (+ per-core bias share). The host sums the four
partials per batch -- the "all-reduce" happens in numpy during unshard, so no
device collective and no duplicated K/V compute (the previous query-split
version duplicated full K/V projections 4x per batch).

Dataflow is fully transposed, no on-chip transposes:
  QT[dh, s] = (Wq_g x^T)/8 + bq_g/8      lhsT = Wq_g^T chunks, rhs = x^T chunk
  KT[dh, s] = Wk_g x^T + bk_g
  V [k, dh] = x Wv_g^T (+ ones column)   lhsT = x^T chunks, rhs = Wv_g^T
  ST[k, q]  = KT_h^T QT_h  (per head, blocked-causal: kt <= 2*qb+1)
  E         = exp(ST + padmask_bias) * causal01 (diag tiles only)
  OT[dh+1,q]= V_aug^T E  (row 64 = softmax denominators)
  AT[dh, q] = OT * (1/denominator)       (broadcast via rank-1 matmul)
  YT[n, q]  = Wo_g^T AT + obias_g        partial; host sums over g
Heads are processed in column-pairs (h, h+2) sharing one [128,512] score/psum
tile (same partition rows, dt 0/1 side by side), two such groups (rows 0-63 /
64-127) per query block. Blocked queries make the causal structure uniform
across cores (only data differs), so SPMD holds. Matmuls run in float32r
(full rate needs free dim >= 256 -- all matmuls use N in {256, 512}).
Softmax skips max-subtraction: scores are bounded (|S| < ~5); masked lanes
get -1e4 pre-exp (padding) or a 0/1 multiply post-exp (causal diagonal).
"""

import sys

if "/opt/trn_rl_repo" not in sys.path:
    sys.path.insert(0, "/opt/trn_rl_repo")

import numpy as np

B, S, D, H, HD = 2, 2048, 1024, 16, 64
N_CORES = 8
G = 4            # head groups (cores per batch)
HG = H // G      # heads per core = 4
DG = D // G      # model dims per core = 256
MC = D // 128    # contraction chunks of 128
NTC = S // 256   # token chunks (proj granularity) = 8
NQB = S // 256   # query blocks = 8
NKT = S // 128   # key tiles = 16

_CACHE = {}


def _split_waits(nc, mybir):
    """This walrus build accepts only one sync-wait per instruction; move
    extra waits onto NOPs inserted just before, on the same engine."""
    n_new = 0
    for f in nc.m.functions:
        for blk in f.blocks:
            out = []
            for inst in blk.instructions:
                si = inst.sync_info
                if si is not None and si.on_wait is not None and len(si.on_wait) > 1:
                    waits = list(si.on_wait)
                    for w in waits[:-1]:
                        n_new += 1
                        out.append(mybir.InstNoOp(
                            name=f"I-waitsplit-{n_new}",
                            engine=inst.engine,
                            ins=[], outs=[],
                            sync_info=mybir.SyncInfo(on_wait=[w], on_update=[]),
                        ))
                    inst.sync_info = mybir.SyncInfo(
                        on_wait=[waits[-1]], on_update=list(si.on_update or []))
                out.append(inst)
            blk.instructions[:] = out
    return n_new


def _build(debug=False):
    import concourse.bass as bass
    import concourse.mybir as mybir
    import concourse.tile as tile
    from contextlib import ExitStack

    f32 = mybir.dt.float32
    f32r = mybir.dt.float32r
    EXP = mybir.ActivationFunctionType.Exp
    MULT = mybir.AluOpType.mult
    ADD = mybir.AluOpType.add

    nc = bass.Bass()
    xT = nc.declare_dram_parameter("xT", [D, S], f32r, isOutput=False)
    wqT = nc.declare_dram_parameter("wqT", [D, DG], f32r, isOutput=False)
    wkT = nc.declare_dram_parameter("wkT", [D, DG], f32r, isOutput=False)
    wvT = nc.declare_dram_parameter("wvT", [D, DG], f32r, isOutput=False)
    woT = nc.declare_dram_parameter("woT", [DG, D], f32r, isOutput=False)
    bq8 = nc.declare_dram_parameter("bq8", [DG], f32, isOutput=False)
    bkv = nc.declare_dram_parameter("bk", [DG], f32, isOutput=False)
    obias = nc.declare_dram_parameter("obias", [D], f32, isOutput=False)
    pmb = nc.declare_dram_parameter("pmb", [S], f32, isOutput=False)
    cmask = nc.declare_dram_parameter("cmask", [2, 128, 512], f32r, isOutput=False)
    onesc = nc.declare_dram_parameter("onesc", [1, 512], f32r, isOutput=False)
    out = nc.declare_dram_parameter("o", [D, S], f32, isOutput=True)
    if debug:
        dbg_e = nc.declare_dram_parameter("dbg_e", [4, 128, 512], f32r,
                                          isOutput=True)
        dbg_rc = nc.declare_dram_parameter("dbg_rc", [2, 1, 512], f32r,
                                           isOutput=True)
        dbg_v = nc.declare_dram_parameter("dbg_v", [128, 2, HG, HD + 1], f32r,
                                          isOutput=True)

    xre = xT.rearrange("(c p) k -> p c k", p=128)
    ore = out.rearrange("(t p) q -> p t q", p=128)

    with tile.TileContext(nc) as tc, ExitStack() as ctx, \
            nc.allow_low_precision("fp32r matmul inputs keep ~19 bits"):
        ec = ctx.enter_context
        consts = ec(tc.tile_pool(name="consts", bufs=1))
        wpool = ec(tc.tile_pool(name="w", bufs=1))
        big = ec(tc.tile_pool(name="big", bufs=1))
        xt_p = ec(tc.tile_pool(name="xt", bufs=2))
        e_p = ec(tc.tile_pool(name="e", bufs=6))
        rc_p = ec(tc.tile_pool(name="rc", bufs=2))
        rb_p = ec(tc.tile_pool(name="rb", bufs=2))
        ots_p = ec(tc.tile_pool(name="ots", bufs=2))
        yt_p = ec(tc.tile_pool(name="yt", bufs=2))
        proj_ps = ec(tc.tile_pool(name="proj_ps", bufs=2, space="PSUM"))
        st_ps = ec(tc.tile_pool(name="st_ps", bufs=2, space="PSUM"))
        ot_ps = ec(tc.tile_pool(name="ot_ps", bufs=4, space="PSUM"))

        # ---- constants + weights into SBUF ----
        # Issue order matters: the sync engine pushes one descriptor at a
        # time (~0.8us each) and the first projection matmul waits on
        # xt0 + wq, so those go first; tensors not needed until the first
        # attention step (cm/pmb) or oproj (wo/ob) are issued after proj(0).
        ones_sb = consts.tile([1, 512], f32r, tag="ones")
        nc.sync.dma_start(out=ones_sb, in_=onesc[:, :])
        xt0_sb = xt_p.tile([128, MC, 256], f32r, tag="xt")
        nc.sync.dma_start(out=xt0_sb, in_=xre[:, :, 0:256])
        wq_sb = wpool.tile([128, MC, DG], f32r, tag="wq")
        nc.sync.dma_start(out=wq_sb, in_=wqT.rearrange("(c p) n -> p c n", p=128))
        bq8_sb = consts.tile([128, 2], f32, tag="bq8")
        nc.sync.dma_start(out=bq8_sb, in_=bq8.rearrange("(c p) -> p c", p=128))
        bk_sb = consts.tile([128, 2], f32, tag="bk")
        nc.sync.dma_start(out=bk_sb, in_=bkv.rearrange("(c p) -> p c", p=128))
        wk_sb = wpool.tile([128, MC, DG], f32r, tag="wk")
        nc.sync.dma_start(out=wk_sb, in_=wkT.rearrange("(c p) n -> p c n", p=128))
        wv_sb = wpool.tile([128, MC, DG], f32r, tag="wv")
        nc.sync.dma_start(out=wv_sb, in_=wvT.rearrange("(c p) n -> p c n", p=128))

        def late_consts():
            cm = consts.tile([128, 2, 512], f32r, tag="cm")
            nc.sync.dma_start(out=cm, in_=cmask.rearrange("t p j -> p t j"))
            pm = consts.tile([128, NKT], f32, tag="pmb")
            nc.sync.dma_start(out=pm, in_=pmb.rearrange("(t p) -> p t", p=128))
            ob = consts.tile([128, MC], f32, tag="ob")
            nc.sync.dma_start(out=ob, in_=obias.rearrange("(c p) -> p c", p=128))
            wo = wpool.tile([128, 2, D], f32r, tag="wo")
            nc.sync.dma_start(out=wo, in_=woT.rearrange("(c p) n -> p c n", p=128))
            return cm, pm, ob, wo

        # persistent activations
        QT_sb = big.tile([128, 2, S], f32r, tag="qt")       # 16KB/part
        KT_sb = big.tile([128, 2, S], f32r, tag="kt")       # 16KB/part
        V_sb = big.tile([128, NKT, HG, HD + 1], f32r, tag="v")
        AT_sb = big.tile([128, 2, S], f32r, tag="at")

        def proj_dma(tc_i):
            """Start the x^T DMA for token chunk tc_i; returns the tile."""
            cols = slice(tc_i * 256, tc_i * 256 + 256)
            xt_sb = xt_p.tile([128, MC, 256], f32r, tag="xt")
            nc.sync.dma_start(out=xt_sb, in_=xre[:, :, cols])
            return xt_sb

        def proj_groups(tc_i, xt_sb):
            """Q/K/V projection matmul groups for token chunk tc_i (256
            tokens), as six independent thunks (PE fillers)."""
            cols = slice(tc_i * 256, tc_i * 256 + 256)

            def qk(dt_, w_sb, dst, is_q):
                ps = proj_ps.tile([128, 256], f32, tag="ps")
                for m in range(MC):
                    nc.tensor.matmul(
                        ps[:], w_sb[:, m, dt_ * 128:dt_ * 128 + 128],
                        xt_sb[:, m, :], start=(m == 0), stop=(m == MC - 1))
                if is_q:
                    nc.vector.tensor_scalar(
                        out=dst[:, dt_, cols], in0=ps[:],
                        scalar1=0.125, scalar2=bq8_sb[:, dt_:dt_ + 1],
                        op0=MULT, op1=ADD)
                else:
                    nc.vector.tensor_scalar_add(
                        out=dst[:, dt_, cols], in0=ps[:],
                        scalar1=bk_sb[:, dt_:dt_ + 1])

            def vproj(kh):
                kt = 2 * tc_i + kh
                ps = proj_ps.tile([128, 256], f32, tag="ps")
                for m in range(MC):
                    nc.tensor.matmul(
                        ps[:], xt_sb[:, m, kh * 128:kh * 128 + 128],
                        wv_sb[:, m, :], start=(m == 0), stop=(m == MC - 1))
                nc.vector.tensor_copy(
                    V_sb[:, kt, :, 0:HD],
                    ps[:].rearrange("p (h d) -> p h d", d=HD))
                # softmax-denominator ones column of V_aug (x*0 + 1)
                nc.vector.tensor_scalar(
                    out=V_sb[:, kt, :, HD:HD + 1],
                    in0=ps[:].rearrange("p (h d) -> p h d", d=HD)[:, :, 0:1],
                    scalar1=0.0, scalar2=1.0, op0=MULT, op1=ADD)

            return [
                lambda d=0: qk(d, wq_sb, QT_sb, True),
                lambda d=1: qk(d, wq_sb, QT_sb, True),
                lambda d=0: qk(d, wk_sb, KT_sb, False),
                lambda d=1: qk(d, wk_sb, KT_sb, False),
                lambda k=0: vproj(k),
                lambda k=1: vproj(k),
            ]

        def attention(qb, fillers):
            """Blocked-causal attention for query block qb (256 queries),
            all 4 heads as two column-pair groups: group r (rows 64r..64r+63)
            covers heads (r, r+2) at dt 0/1. `fillers` is a list of thunks
            (independent PE matmul groups -- next chunk's projections, earlier
            blocks' output projection) woven between the score matmuls and the
            exp-dependent PV matmuls so the in-order PE queue never stalls on
            the Scalar engine."""
            nkt = 2 * qb + 2
            qcols = slice(qb * 256, qb * 256 + 256)
            # one accumulator tile per (group, dt): interleaved open
            # accumulation groups must not share a PSUM bank
            ot_a0 = ot_ps.tile([HD + 1, 256], f32, tag="ot")
            ot_a1 = ot_ps.tile([HD + 1, 256], f32, tag="ot")
            ot_b0 = ot_ps.tile([HD + 1, 256], f32, tag="ot")
            ot_b1 = ot_ps.tile([HD + 1, 256], f32, tag="ot")
            ots = [[ot_a0, ot_a1], [ot_b0, ot_b1]]
            nfill = len(fillers)
            fi = 0
            for kt in range(nkt):
                kcols = slice(kt * 128, kt * 128 + 128)
                es = []
                for r in range(2):
                    r0 = 64 * r
                    st = st_ps.tile([128, 512], f32, tag="st")
                    for dt_ in range(2):
                        nc.tensor.matmul(
                            st[:, dt_ * 256:dt_ * 256 + 256],
                            KT_sb[r0:r0 + 64, dt_, kcols],
                            QT_sb[r0:r0 + 64, dt_, qcols],
                            start=True, stop=True)
                    e = e_p.tile([128, 512], f32r, tag="e")
                    nc.scalar.activation(out=e[:], in_=st[:], func=EXP,
                                         bias=pmb_sb[:, kt:kt + 1])
                    if kt >= 2 * qb:
                        nc.gpsimd.tensor_mul(e[:], e[:],
                                             cm_sb[:, kt - 2 * qb, :])
                    if debug and qb == 0:
                        nc.sync.dma_start(out=dbg_e[2 * kt + r], in_=e[:])
                    es.append(e)
                # evenly spread fillers across rounds (PE work between the
                # score matmuls and the exp-dependent PV matmuls)
                while fi * nkt < nfill * (kt + 1):
                    fillers[fi]()
                    fi += 1
                for r in range(2):
                    for dt_ in range(2):
                        h = r + 2 * dt_
                        nc.tensor.matmul(
                            ots[r][dt_][:],
                            V_sb[:, kt, h, :],
                            es[r][:, dt_ * 256:dt_ * 256 + 256],
                            start=(kt == 0), stop=(kt == nkt - 1))
            # normalize: AT = ot[0:64] / ot[64]. Stage the four accumulators
            # into one SBUF tile first -- that releases the PSUM banks ~1.5us
            # after the last PV so the next query block's PV never stalls on
            # this block's (slow, 3.3us) reciprocal. One batched reciprocal,
            # denominator-reciprocal broadcast by rank-1 matmul, two muls.
            ot_s = ots_p.tile([HD + 1, 4, 256], f32, tag="ots")
            for r in range(2):
                for dt_ in range(2):
                    nc.vector.tensor_copy(ot_s[:, 2 * r + dt_, :],
                                          ots[r][dt_][:])
            rcf = rc_p.tile([1, 4, 256], f32, tag="rcf")
            nc.vector.reciprocal(out=rcf[:], in_=ot_s[HD:HD + 1, :, :])
            rcr = rc_p.tile([1, 4, 256], f32r, tag="rcr")
            nc.vector.tensor_copy(rcr[:], rcf[:])
            if debug and qb == 0:
                nc.sync.dma_start(out=dbg_rc[:, :, :],
                                  in_=rcr.rearrange("o (t j) -> o t j", t=2))
            bc_a = st_ps.tile([HD, 512], f32, tag="st")
            nc.tensor.matmul(bc_a[:], ones_sb[:, 0:HD],
                             rcr[:, 0:2, :].rearrange("o t j -> o (t j)"),
                             start=True, stop=True)
            bc_b = st_ps.tile([HD, 512], f32, tag="st")
            nc.tensor.matmul(bc_b[:], ones_sb[:, 0:HD],
                             rcr[:, 2:4, :].rearrange("o t j -> o (t j)"),
                             start=True, stop=True)
            rb_a = rb_p.tile([HD, 512], f32, tag="rba")
            nc.vector.tensor_copy(rb_a[:], bc_a[:])
            rb_b = rb_p.tile([HD, 512], f32, tag="rbb")
            nc.vector.tensor_copy(rb_b[:], bc_b[:])
            for r, rb in ((0, rb_a), (1, rb_b)):
                r0 = 64 * r
                nc.vector.tensor_mul(
                    AT_sb[r0:r0 + 64, :, qcols],
                    ot_s[0:HD, 2 * r:2 * r + 2, :],
                    rb[:].rearrange("p (t j) -> p t j", j=256))

        def oproj_groups(qb):
            """Row-parallel partial output projection for query block qb, as
            eight single-output-tile thunks (PE fillers)."""
            qcols = slice(qb * 256, qb * 256 + 256)
            yt = yt_p.tile([128, MC, 256], f32, tag="yt")

            def one(nt):
                ps = proj_ps.tile([128, 256], f32, tag="ps")
                for c in range(2):
                    nc.tensor.matmul(
                        ps[:], wo_sb[:, c, nt * 128:nt * 128 + 128],
                        AT_sb[:, c, qcols], start=(c == 0), stop=(c == 1))
                nc.vector.tensor_scalar_add(
                    out=yt[:, nt, :], in0=ps[:], scalar1=ob_sb[:, nt:nt + 1])
                nc.sync.dma_start(out=ore[:, nt, qcols], in_=yt[:, nt, :])

            return [lambda n=n: one(n) for n in range(MC)]

        # ---- pipelined schedule ----
        # step t: attention(t-1) with proj(t) + oproj(t-2) groups as PE
        # fillers between score and PV matmuls; x^T chunk t+1 prefetched.
        xt_cur = xt0_sb
        # HAM warm-up: the PE clock gate defaults to 4/8 (1.2 GHz) and takes
        # ~3.4us of sustained activity to open. The first ~15-25us of the
        # kernel are DMA preamble with an idle PE, so burn that wait on dummy
        # matmuls (ones x ones into a scratch psum bank that is never read):
        # the real work then starts at 2.4 GHz.
        warm = st_ps.tile([HD, 512], f32, tag="st")
        for i in range(40):
            nc.tensor.matmul(warm[:], ones_sb[:, 0:HD], ones_sb[:],
                             start=True, stop=True)
        for grp in proj_groups(0, xt_cur):
            grp()
        cm_sb, pmb_sb, ob_sb, wo_sb = late_consts()
        xt_cur = proj_dma(1)
        for t in range(1, NTC):
            fillers = []
            if t + 1 < NTC:
                xt_nxt = []
                fillers.append(lambda tn=t + 1, h=xt_nxt: h.append(proj_dma(tn)))
            fillers += proj_groups(t, xt_cur)
            if t >= 2:
                fillers += oproj_groups(t - 2)
            attention(t - 1, fillers)
            if t + 1 < NTC:
                xt_cur = xt_nxt[0]
        attention(NQB - 1, oproj_groups(NQB - 2))
        for grp in oproj_groups(NQB - 1):
            grp()
        if debug:
            nc.sync.dma_start(out=dbg_v[:, :, :, :], in_=V_sb[:, 0:2, :, :])

    _split_waits(nc, mybir)
    return nc


def _get_nc():
    if "nc" not in _CACHE:
        _CACHE["nc"] = _build()
    return _CACHE["nc"]


def _make_inputs(x, mask, Wq, bq, Wk, bk, Wv, bv, Wo, bo):
    f = np.float32
    x = np.asarray(x, f)
    mask = np.asarray(mask)
    Wq, bq = np.asarray(Wq, f), np.asarray(bq, f)
    Wk, bk = np.asarray(Wk, f), np.asarray(bk, f)
    Wv, bv = np.asarray(Wv, f), np.asarray(bv, f)
    Wo, bo = np.asarray(Wo, f), np.asarray(bo, f)

    wqT = np.ascontiguousarray(Wq.T)
    wkT = np.ascontiguousarray(Wk.T)
    wvT = np.ascontiguousarray(Wv.T)
    woT = np.ascontiguousarray(Wo.T)

    xTb = [np.ascontiguousarray(x[b].T) for b in range(B)]
    pmbb = [((mask[b].astype(f) - 1.0) * 1e4).astype(f) for b in range(B)]

    kk, qq = np.meshgrid(np.arange(128), np.arange(256), indexing="ij")
    cm = np.empty((2, 128, 512), f)
    cm[0, :, 0:256] = (kk <= qq).astype(f)
    cm[0, :, 256:512] = cm[0, :, 0:256]
    cm[1, :, 0:256] = (kk + 128 <= qq).astype(f)
    cm[1, :, 256:512] = cm[1, :, 0:256]
    onesc = np.ones((1, 512), f)

    ins = []
    for c in range(N_CORES):
        b, g = c // 4, c % 4
        dlo = DG * g
        obias_c = (Wo[:, dlo:dlo + DG] @ bv[dlo:dlo + DG]).astype(f)
        if g == 0:
            obias_c = (obias_c + bo).astype(f)
        ins.append({
            "xT": xTb[b],
            "wqT": np.ascontiguousarray(wqT[:, dlo:dlo + DG]),
            "wkT": np.ascontiguousarray(wkT[:, dlo:dlo + DG]),
            "wvT": np.ascontiguousarray(wvT[:, dlo:dlo + DG]),
            "woT": np.ascontiguousarray(woT[dlo:dlo + DG, :]),
            "bq8": (bq[dlo:dlo + DG] / 8.0).astype(f),
            "bk": np.ascontiguousarray(bk[dlo:dlo + DG]),
            "obias": obias_c,
            "pmb": pmbb[b],
            "cmask": cm,
            "onesc": onesc,
        })
    return ins


def _run(ins, trace=False):
    from concourse.bass_utils import run_bass_kernel_spmd
    nc = _get_nc()
    return run_bass_kernel_spmd(nc, ins, list(range(N_CORES)), trace=trace)


def kernel(x, mask, Wq, bq, Wk, bk, Wv, bv, Wo, bo):
    ins = _make_inputs(x, mask, Wq, bq, Wk, bk, Wv, bv, Wo, bo)
    res = _run(ins)
    out = np.zeros((B, S, D), np.float32)
    for c in range(N_CORES):
        b = c // 4
        out[b] += res.results[c]["o"].T
    return out


# revision 45
# speedup vs baseline: 1.3085x; 1.3085x over previous
"""Multi-head self-attention (B=2, S=2048, D=1024, H=16, causal+padding mask)
on 8 Trainium2 NeuronCores via Bass/Tile, SPMD.

Sharding: head-split tensor parallelism. Core c -> batch b = c//4, head group
g = c%4 (heads 4g..4g+3, model dims 256g..256g+255). Each core projects only
its 256-dim Q/K/V slice over the full 2048-token batch, runs blocked-causal
attention for its 4 heads, and emits a row-parallel partial output
Y_g^T = Wo[:, g-dims]^T A_g^T (+ per-core bias share). The host sums the four
partials per batch -- the "all-reduce" happens in numpy during unshard, so no
device collective and no duplicated K/V compute (the previous query-split
version duplicated full K/V projections 4x per batch).

Dataflow is fully transposed, no on-chip transposes:
  QT[dh, s] = (Wq_g x^T)/8 + bq_g/8      lhsT = Wq_g^T chunks, rhs = x^T chunk
  KT[dh, s] = Wk_g x^T + bk_g
  V [k, dh] = x Wv_g^T (+ ones column)   lhsT = x^T chunks, rhs = Wv_g^T
  ST[k, q]  = KT_h^T QT_h  (per head, blocked-causal: kt <= 2*qb+1)
  E         = exp(ST + padmask_bias) * causal01 (diag tiles only)
  OT[dh+1,q]= V_aug^T E  (row 64 = softmax denominators)
  AT[dh, q] = OT * (1/denominator)       (broadcast via rank-1 matmul)
  YT[n, q]  = Wo_g^T AT + obias_g        partial; host sums over g
Heads are processed in column-pairs (h, h+2) sharing one [128,512] score/psum
tile (same partition rows, dt 0/1 side by side), two such groups (rows 0-63 /
64-127) per query block. Blocked queries make the causal structure uniform
across cores (only data differs), so SPMD holds. Matmuls run in float32r
(full rate needs free dim >= 256 -- all matmuls use N in {256, 512}).
Softmax skips max-subtraction: scores are bounded (|S| < ~5); masked lanes
get -1e4 pre-exp (padding) or a 0/1 multiply post-exp (causal diagonal).
"""

import sys

if "/opt/trn_rl_repo" not in sys.path:
    sys.path.insert(0, "/opt/trn_rl_repo")

import numpy as np

B, S, D, H, HD = 2, 2048, 1024, 16, 64
N_CORES = 8
G = 4            # head groups (cores per batch)
HG = H // G      # heads per core = 4
DG = D // G      # model dims per core = 256
MC = D // 128    # contraction chunks of 128
NTC = S // 256   # token chunks (proj granularity) = 8
NQB = S // 256   # query blocks = 8
NKT = S // 128   # key tiles = 16

_CACHE = {}


def _split_waits(nc, mybir):
    """This walrus build accepts only one sync-wait per instruction; move
    extra waits onto NOPs inserted just before, on the same engine."""
    n_new = 0
    for f in nc.m.functions:
        for blk in f.blocks:
            out = []
            for inst in blk.instructions:
                si = inst.sync_info
                if si is not None and si.on_wait is not None and len(si.on_wait) > 1:
                    waits = list(si.on_wait)
                    for w in waits[:-1]:
                        n_new += 1
                        out.append(mybir.InstNoOp(
                            name=f"I-waitsplit-{n_new}",
                            engine=inst.engine,
                            ins=[], outs=[],
                            sync_info=mybir.SyncInfo(on_wait=[w], on_update=[]),
                        ))
                    inst.sync_info = mybir.SyncInfo(
                        on_wait=[waits[-1]], on_update=list(si.on_update or []))
                out.append(inst)
            blk.instructions[:] = out
    return n_new


def _build(debug=False):
    import concourse.bass as bass
    import concourse.mybir as mybir
    import concourse.tile as tile
    from contextlib import ExitStack

    f32 = mybir.dt.float32
    f32r = mybir.dt.float32r
    EXP = mybir.ActivationFunctionType.Exp
    LN = mybir.ActivationFunctionType.Ln
    MULT = mybir.AluOpType.mult
    ADD = mybir.AluOpType.add

    nc = bass.Bass()
    xT = nc.declare_dram_parameter("xT", [D, S], f32r, isOutput=False)
    wqT = nc.declare_dram_parameter("wqT", [D, DG], f32r, isOutput=False)
    wkT = nc.declare_dram_parameter("wkT", [D, DG], f32r, isOutput=False)
    wvT = nc.declare_dram_parameter("wvT", [D, DG], f32r, isOutput=False)
    woT = nc.declare_dram_parameter("woT", [DG, D], f32r, isOutput=False)
    bq8 = nc.declare_dram_parameter("bq8", [DG], f32, isOutput=False)
    bkv = nc.declare_dram_parameter("bk", [DG], f32, isOutput=False)
    obias = nc.declare_dram_parameter("obias", [D], f32, isOutput=False)
    pmb = nc.declare_dram_parameter("pmb", [S], f32, isOutput=False)
    cmask = nc.declare_dram_parameter("cmask", [2, 128, 512], f32r, isOutput=False)
    onesc = nc.declare_dram_parameter("onesc", [1, 512], f32r, isOutput=False)
    out = nc.declare_dram_parameter("o", [D, S], f32, isOutput=True)
    if debug:
        dbg_e = nc.declare_dram_parameter("dbg_e", [4, 128, 512], f32r,
                                          isOutput=True)
        dbg_rc = nc.declare_dram_parameter("dbg_rc", [2, 1, 512], f32r,
                                           isOutput=True)
        dbg_v = nc.declare_dram_parameter("dbg_v", [128, 2, HG, HD + 1], f32r,
                                          isOutput=True)

    xre = xT.rearrange("(c p) k -> p c k", p=128)
    ore = out.rearrange("(t p) q -> p t q", p=128)

    with tile.TileContext(nc) as tc, ExitStack() as ctx, \
            nc.allow_low_precision("fp32r matmul inputs keep ~19 bits"):
        ec = ctx.enter_context
        consts = ec(tc.tile_pool(name="consts", bufs=1))
        wpool = ec(tc.tile_pool(name="w", bufs=1))
        big = ec(tc.tile_pool(name="big", bufs=1))
        xt_p = ec(tc.tile_pool(name="xt", bufs=2))
        e_p = ec(tc.tile_pool(name="e", bufs=6))
        rc_p = ec(tc.tile_pool(name="rc", bufs=2))
        rb_p = ec(tc.tile_pool(name="rb", bufs=2))
        ots_p = ec(tc.tile_pool(name="ots", bufs=2))
        yt_p = ec(tc.tile_pool(name="yt", bufs=2))
        proj_ps = ec(tc.tile_pool(name="proj_ps", bufs=2, space="PSUM"))
        st_ps = ec(tc.tile_pool(name="st_ps", bufs=2, space="PSUM"))
        ot_ps = ec(tc.tile_pool(name="ot_ps", bufs=4, space="PSUM"))

        # ---- constants + weights into SBUF ----
        # Issue order matters: the sync engine pushes one descriptor at a
        # time (~0.8us each) and the first projection matmul waits on
        # xt0 + wq, so those go first; tensors not needed until the first
        # attention step (cm/pmb) or oproj (wo/ob) are issued after proj(0).
        ones_sb = consts.tile([1, 512], f32r, tag="ones")
        nc.sync.dma_start(out=ones_sb, in_=onesc[:, :])
        xt0_sb = xt_p.tile([128, MC, 256], f32r, tag="xt")
        nc.sync.dma_start(out=xt0_sb, in_=xre[:, :, 0:256])
        wq_sb = wpool.tile([128, MC, DG], f32r, tag="wq")
        nc.sync.dma_start(out=wq_sb, in_=wqT.rearrange("(c p) n -> p c n", p=128))
        bq8_sb = consts.tile([128, 2], f32, tag="bq8")
        nc.sync.dma_start(out=bq8_sb, in_=bq8.rearrange("(c p) -> p c", p=128))
        bk_sb = consts.tile([128, 2], f32, tag="bk")
        nc.sync.dma_start(out=bk_sb, in_=bkv.rearrange("(c p) -> p c", p=128))
        wk_sb = wpool.tile([128, MC, DG], f32r, tag="wk")
        nc.sync.dma_start(out=wk_sb, in_=wkT.rearrange("(c p) n -> p c n", p=128))
        wv_sb = wpool.tile([128, MC, DG], f32r, tag="wv")
        nc.sync.dma_start(out=wv_sb, in_=wvT.rearrange("(c p) n -> p c n", p=128))

        def late_consts():
            cm = consts.tile([128, 2, 512], f32r, tag="cm")
            nc.sync.dma_start(out=cm, in_=cmask.rearrange("t p j -> p t j"))
            pm = consts.tile([128, NKT], f32, tag="pmb")
            nc.sync.dma_start(out=pm, in_=pmb.rearrange("(t p) -> p t", p=128))
            ob = consts.tile([128, MC], f32, tag="ob")
            nc.sync.dma_start(out=ob, in_=obias.rearrange("(c p) -> p c", p=128))
            wo = wpool.tile([128, 2, D], f32r, tag="wo")
            nc.sync.dma_start(out=wo, in_=woT.rearrange("(c p) n -> p c n", p=128))
            return cm, pm, ob, wo

        # persistent activations
        QT_sb = big.tile([128, 2, S], f32r, tag="qt")       # 16KB/part
        KT_sb = big.tile([128, 2, S], f32r, tag="kt")       # 16KB/part
        V_sb = big.tile([128, NKT, HG, HD + 1], f32r, tag="v")
        AT_sb = big.tile([128, 2, S], f32r, tag="at")

        def proj_dma(tc_i):
            """Start the x^T DMA for token chunk tc_i; returns the tile."""
            cols = slice(tc_i * 256, tc_i * 256 + 256)
            xt_sb = xt_p.tile([128, MC, 256], f32r, tag="xt")
            nc.sync.dma_start(out=xt_sb, in_=xre[:, :, cols])
            return xt_sb

        def proj_groups(tc_i, xt_sb):
            """Q/K/V projection matmul groups for token chunk tc_i (256
            tokens), as six independent thunks (PE fillers)."""
            cols = slice(tc_i * 256, tc_i * 256 + 256)

            def qk(dt_, w_sb, dst, is_q):
                ps = proj_ps.tile([128, 256], f32, tag="ps")
                for m in range(MC):
                    nc.tensor.matmul(
                        ps[:], w_sb[:, m, dt_ * 128:dt_ * 128 + 128],
                        xt_sb[:, m, :], start=(m == 0), stop=(m == MC - 1))
                if is_q:
                    nc.vector.tensor_scalar(
                        out=dst[:, dt_, cols], in0=ps[:],
                        scalar1=0.125, scalar2=bq8_sb[:, dt_:dt_ + 1],
                        op0=MULT, op1=ADD)
                else:
                    nc.vector.tensor_scalar_add(
                        out=dst[:, dt_, cols], in0=ps[:],
                        scalar1=bk_sb[:, dt_:dt_ + 1])

            def vproj(kh):
                kt = 2 * tc_i + kh
                ps = proj_ps.tile([128, 256], f32, tag="ps")
                for m in range(MC):
                    nc.tensor.matmul(
                        ps[:], xt_sb[:, m, kh * 128:kh * 128 + 128],
                        wv_sb[:, m, :], start=(m == 0), stop=(m == MC - 1))
                nc.vector.tensor_copy(
                    V_sb[:, kt, :, 0:HD],
                    ps[:].rearrange("p (h d) -> p h d", d=HD))
                # softmax-denominator ones column of V_aug (x*0 + 1)
                nc.vector.tensor_scalar(
                    out=V_sb[:, kt, :, HD:HD + 1],
                    in0=ps[:].rearrange("p (h d) -> p h d", d=HD)[:, :, 0:1],
                    scalar1=0.0, scalar2=1.0, op0=MULT, op1=ADD)

            return [
                lambda d=0: qk(d, wq_sb, QT_sb, True),
                lambda d=1: qk(d, wq_sb, QT_sb, True),
                lambda d=0: qk(d, wk_sb, KT_sb, False),
                lambda d=1: qk(d, wk_sb, KT_sb, False),
                lambda k=0: vproj(k),
                lambda k=1: vproj(k),
            ]

        def attention(qb, fillers):
            """Blocked-causal attention for query block qb (256 queries),
            all 4 heads as two column-pair groups: group r (rows 64r..64r+63)
            covers heads (r, r+2) at dt 0/1. `fillers` is a list of thunks
            (independent PE matmul groups -- next chunk's projections, earlier
            blocks' output projection) woven between the score matmuls and the
            exp-dependent PV matmuls so the in-order PE queue never stalls on
            the Scalar engine."""
            nkt = 2 * qb + 2
            qcols = slice(qb * 256, qb * 256 + 256)
            # one accumulator tile per (group, dt): interleaved open
            # accumulation groups must not share a PSUM bank
            ot_a0 = ot_ps.tile([HD + 1, 256], f32, tag="ot")
            ot_a1 = ot_ps.tile([HD + 1, 256], f32, tag="ot")
            ot_b0 = ot_ps.tile([HD + 1, 256], f32, tag="ot")
            ot_b1 = ot_ps.tile([HD + 1, 256], f32, tag="ot")
            ots = [[ot_a0, ot_a1], [ot_b0, ot_b1]]
            nfill = len(fillers)
            fi = 0
            for kt in range(nkt):
                kcols = slice(kt * 128, kt * 128 + 128)
                es = []
                for r in range(2):
                    r0 = 64 * r
                    st = st_ps.tile([128, 512], f32, tag="st")
                    for dt_ in range(2):
                        nc.tensor.matmul(
                            st[:, dt_ * 256:dt_ * 256 + 256],
                            KT_sb[r0:r0 + 64, dt_, kcols],
                            QT_sb[r0:r0 + 64, dt_, qcols],
                            start=True, stop=True)
                    e = e_p.tile([128, 512], f32r, tag="e")
                    nc.scalar.activation(out=e[:], in_=st[:], func=EXP,
                                         bias=pmb_sb[:, kt:kt + 1])
                    if kt >= 2 * qb:
                        nc.gpsimd.tensor_mul(e[:], e[:],
                                             cm_sb[:, kt - 2 * qb, :])
                    if debug and qb == 0:
                        nc.sync.dma_start(out=dbg_e[2 * kt + r], in_=e[:])
                    es.append(e)
                # evenly spread fillers across rounds (PE work between the
                # score matmuls and the exp-dependent PV matmuls)
                while fi * nkt < nfill * (kt + 1):
                    fillers[fi]()
                    fi += 1
                for r in range(2):
                    for dt_ in range(2):
                        h = r + 2 * dt_
                        nc.tensor.matmul(
                            ots[r][dt_][:],
                            V_sb[:, kt, h, :],
                            es[r][:, dt_ * 256:dt_ * 256 + 256],
                            start=(kt == 0), stop=(kt == nkt - 1))
            # normalize: AT = ot[0:64] / ot[64]. Stage the four accumulators
            # into one SBUF tile first -- that releases the PSUM banks ~1.5us
            # after the last PV so the next query block's PV never stalls on
            # this block's (slow, 3.3us) reciprocal. One batched reciprocal,
            # denominator-reciprocal broadcast by rank-1 matmul, two muls.
            ot_s = ots_p.tile([HD + 1, 4, 256], f32, tag="ots")
            for r in range(2):
                for dt_ in range(2):
                    nc.vector.tensor_copy(ot_s[:, 2 * r + dt_, :],
                                          ots[r][dt_][:])
            # 1/denominator as exp(-ln(d)) on the Scalar engine: ln and exp
            # share an ACT table set (no reloads), each call is ~1.1us on
            # [1,1024], and -- unlike DVE reciprocal (3-6us, serial over the
            # free dim) -- nothing on the PE or DVE queue waits on it.
            lnd = rc_p.tile([1, 4, 256], f32, tag="lnd")
            nc.scalar.activation(out=lnd[:], in_=ot_s[HD:HD + 1, :, :],
                                 func=LN)
            rr = rc_p.tile([1, 4, 256], f32r, tag="rr")
            nc.scalar.activation(out=rr[:], in_=lnd[:], func=EXP, scale=-1.0)
            if debug and qb == 0:
                nc.sync.dma_start(out=dbg_rc[:, :, :],
                                  in_=rr.rearrange("o (t j) -> o t j", t=2))

            def norm_fin():
                # broadcast 1/d by rank-1 matmul, then scale the attention
                # output. Deferred two steps as a PE filler so the bc
                # matmuls never wait on the reciprocal chain.
                bc_a = st_ps.tile([HD, 512], f32, tag="st")
                nc.tensor.matmul(bc_a[:], ones_sb[:, 0:HD],
                                 rr[:, 0:2, :].rearrange("o t j -> o (t j)"),
                                 start=True, stop=True)
                bc_b = st_ps.tile([HD, 512], f32, tag="st")
                nc.tensor.matmul(bc_b[:], ones_sb[:, 0:HD],
                                 rr[:, 2:4, :].rearrange("o t j -> o (t j)"),
                                 start=True, stop=True)
                rb_a = rb_p.tile([HD, 512], f32, tag="rba")
                nc.vector.tensor_copy(rb_a[:], bc_a[:])
                rb_b = rb_p.tile([HD, 512], f32, tag="rbb")
                nc.vector.tensor_copy(rb_b[:], bc_b[:])
                for r, rb in ((0, rb_a), (1, rb_b)):
                    r0 = 64 * r
                    nc.vector.tensor_mul(
                        AT_sb[r0:r0 + 64, :, qcols],
                        ot_s[0:HD, 2 * r:2 * r + 2, :],
                        rb[:].rearrange("p (t j) -> p t j", j=256))

            return norm_fin

        def oproj_groups(qb):
            """Row-parallel partial output projection for query block qb, as
            eight single-output-tile thunks (PE fillers)."""
            qcols = slice(qb * 256, qb * 256 + 256)
            yt = yt_p.tile([128, MC, 256], f32, tag="yt")

            def one(nt):
                ps = proj_ps.tile([128, 256], f32, tag="ps")
                for c in range(2):
                    nc.tensor.matmul(
                        ps[:], wo_sb[:, c, nt * 128:nt * 128 + 128],
                        AT_sb[:, c, qcols], start=(c == 0), stop=(c == 1))
                nc.vector.tensor_scalar_add(
                    out=yt[:, nt, :], in0=ps[:], scalar1=ob_sb[:, nt:nt + 1])
                nc.sync.dma_start(out=ore[:, nt, qcols], in_=yt[:, nt, :])

            return [lambda n=n: one(n) for n in range(MC)]

        # ---- pipelined schedule ----
        # step t: attention(t-1) with proj(t) + oproj(t-2) groups as PE
        # fillers between score and PV matmuls; x^T chunk t+1 prefetched.
        xt_cur = xt0_sb
        # HAM warm-up: the PE clock gate defaults to 4/8 (1.2 GHz) and takes
        # ~3.4us of sustained activity to open. The first ~15-25us of the
        # kernel are DMA preamble with an idle PE, so burn that wait on dummy
        # matmuls (ones x ones into a scratch psum bank that is never read):
        # the real work then starts at 2.4 GHz.
        warm = st_ps.tile([HD, 512], f32, tag="st")
        for i in range(40):
            nc.tensor.matmul(warm[:], ones_sb[:, 0:HD], ones_sb[:],
                             start=True, stop=True)
        for grp in proj_groups(0, xt_cur):
            grp()
        cm_sb, pmb_sb, ob_sb, wo_sb = late_consts()
        xt_cur = proj_dma(1)
        norm_fins = {}
        for t in range(1, NTC):
            fillers = []
            if t + 1 < NTC:
                xt_nxt = []
                fillers.append(lambda tn=t + 1, h=xt_nxt: h.append(proj_dma(tn)))
            fillers += proj_groups(t, xt_cur)
            if t >= 2:
                fillers.append(norm_fins.pop(t - 2))
                fillers += oproj_groups(t - 2)
            norm_fins[t - 1] = attention(t - 1, fillers)
            if t + 1 < NTC:
                xt_cur = xt_nxt[0]
        last_fill = [norm_fins.pop(NQB - 2)] + oproj_groups(NQB - 2)
        norm_fins[NQB - 1] = attention(NQB - 1, last_fill)
        norm_fins.pop(NQB - 1)()
        for grp in oproj_groups(NQB - 1):
            grp()
        if debug:
            nc.sync.dma_start(out=dbg_v[:, :, :, :], in_=V_sb[:, 0:2, :, :])

    _split_waits(nc, mybir)
    return nc


def _get_nc():
    if "nc" not in _CACHE:
        _CACHE["nc"] = _build()
    return _CACHE["nc"]


def _make_inputs(x, mask, Wq, bq, Wk, bk, Wv, bv, Wo, bo):
    f = np.float32
    x = np.asarray(x, f)
    mask = np.asarray(mask)
    Wq, bq = np.asarray(Wq, f), np.asarray(bq, f)
    Wk, bk = np.asarray(Wk, f), np.asarray(bk, f)
    Wv, bv = np.asarray(Wv, f), np.asarray(bv, f)
    Wo, bo = np.asarray(Wo, f), np.asarray(bo, f)

    wqT = np.ascontiguousarray(Wq.T)
    wkT = np.ascontiguousarray(Wk.T)
    wvT = np.ascontiguousarray(Wv.T)
    woT = np.ascontiguousarray(Wo.T)

    xTb = [np.ascontiguousarray(x[b].T) for b in range(B)]
    pmbb = [((mask[b].astype(f) - 1.0) * 1e4).astype(f) for b in range(B)]

    kk, qq = np.meshgrid(np.arange(128), np.arange(256), indexing="ij")
    cm = np.empty((2, 128, 512), f)
    cm[0, :, 0:256] = (kk <= qq).astype(f)
    cm[0, :, 256:512] = cm[0, :, 0:256]
    cm[1, :, 0:256] = (kk + 128 <= qq).astype(f)
    cm[1, :, 256:512] = cm[1, :, 0:256]
    onesc = np.ones((1, 512), f)

    ins = []
    for c in range(N_CORES):
        b, g = c // 4, c % 4
        dlo = DG * g
        obias_c = (Wo[:, dlo:dlo + DG] @ bv[dlo:dlo + DG]).astype(f)
        if g == 0:
            obias_c = (obias_c + bo).astype(f)
        ins.append({
            "xT": xTb[b],
            "wqT": np.ascontiguousarray(wqT[:, dlo:dlo + DG]),
            "wkT": np.ascontiguousarray(wkT[:, dlo:dlo + DG]),
            "wvT": np.ascontiguousarray(wvT[:, dlo:dlo + DG]),
            "woT": np.ascontiguousarray(woT[dlo:dlo + DG, :]),
            "bq8": (bq[dlo:dlo + DG] / 8.0).astype(f),
            "bk": np.ascontiguousarray(bk[dlo:dlo + DG]),
            "obias": obias_c,
            "pmb": pmbb[b],
            "cmask": cm,
            "onesc": onesc,
        })
    return ins


def _run(ins, trace=False):
    from concourse.bass_utils import run_bass_kernel_spmd
    nc = _get_nc()
    return run_bass_kernel_spmd(nc, ins, list(range(N_CORES)), trace=trace)


def kernel(x, mask, Wq, bq, Wk, bk, Wv, bv, Wo, bo):
    ins = _make_inputs(x, mask, Wq, bq, Wk, bk, Wv, bv, Wo, bo)
    res = _run(ins)
    out = np.zeros((B, S, D), np.float32)
    for c in range(N_CORES):
        b = c // 4
        out[b] += res.results[c]["o"].T
    return out
